# revision 8
# baseline (speedup 1.0000x reference)
"""MCR2 variational loss on 8 Trainium2 NeuronCores.

Strategy (data-parallel over the sample axis n):
  - The heavy part of the loss is the per-class second-moment matrices
    M_j = Z^T diag(Pi_j) Z (plus the global gram Z^T Z), which reads all of
    Z/Pi once -> memory-bound. Everything downstream (logdet, log1p terms,
    Frobenius distance) is O(C*d^2) scalar work done on the host in fp64.
  - Fast path (Pi exactly one-hot): each sample contributes to exactly one
    class, so per-class partial grams over class-sorted rows give all M_j,
    and gram = sum_j M_j. Host distributes rows so every core gets an
    almost equal share of each class, pads each class segment to a 128-row
    multiple, and the device accumulates each class's gram in its PSUM
    slice: fp8e4m3 DoubleRow matmuls crunch 256 rows per instruction (two
    128-row subchunks packed per partition), with one plain fp8 matmul for
    a class's odd trailing subchunk. fp8 keeps the final losses within
    ~1.6e-3 relative (measured), far inside the 2e-2 gate, while quartering
    HBM traffic vs fp32 and doubling PE throughput.
  - DMA: the 16 SDMA engines round-robin packets across the two HWDGE
    rings, and each per-partition contiguous run is one descriptor, so
    aggregate input bandwidth scales with descriptor size (~105ns fixed +
    bytes/27GB/s per descriptor per engine). Tiles therefore ramp from 6
    subchunks (768B descriptors, fast first-tile latency so the PE starts
    early) up to 26 subchunks (3.3KB descriptors, ~230GB/s aggregate),
    with a small final tile so the PE tail isn't gated on a huge transfer.
    Rings alternate strictly (sync/scalar); no SWDGE in the data path
    keeps the Pool engine's exit drain off the critical path.
  - Output: classes 0-8 are drained PSUM->SBUF (fp16 cast) per bank group
    and stored on the scalar ring while the PE still works; only class 9's
    256B/partition store (sync ring) trails the last matmul.
  - Fallback (general dense Pi): host BLAS contraction.
"""

import numpy as np

EPS = 0.5
MU = 1.0
C = 10
N_TOTAL = 131072
D = 128
N_CORES = 8
CHUNK = 128  # rows per subchunk (PE partition/contraction dim)

_compiled_cache = {}


def _matmul_plan(seg_sub):
    """Per-class unit decomposition: DoubleRow units of 2 subchunks first,
    then a plain single-subchunk matmul when the class length is odd. Each
    class's PSUM accumulation group stays CONTIGUOUS in the instruction
    stream — interleaving groups (e.g. starting all classes up front)
    corrupts earlier partial sums on hardware (measured 9e-2 error).

    Returns (plan, plain_pos, dr_pos): plan is a position-sorted list of
    (global_subchunk_pos, size, class, is_first, is_last); plain_pos maps
    class -> subchunk index of its plain chunk; dr_pos maps class -> first
    subchunk of its DoubleRow block."""
    plain_pos = {}
    dr_pos = {}
    plan = []
    pos = 0
    for j, s in enumerate(seg_sub):
        ndr = s // 2
        dr_pos[j] = pos
        for u in range(ndr):
            plan.append((pos + 2 * u, 2, j, u == 0, s % 2 == 0 and u == ndr - 1))
        if s % 2 == 1:
            plain_pos[j] = pos + 2 * ndr
            plan.append((plain_pos[j], 1, j, ndr == 0, True))
        pos += s
    return plan, plain_pos, dr_pos


def _dma_tile_sizes(seg_sub):
    """Ramped tile sizes (in subchunks). Boundaries never split a DoubleRow
    unit: legal cut points within class c (start s0, length s) are
    s0+{0,2,..} and s0+s. Ramp: small first tiles for PE start latency,
    3-4KB descriptors in the middle for DMA bandwidth, small last tile so
    the PE tail isn't gated on a monster transfer."""
    total = sum(seg_sub)
    legal = set()
    pos = 0
    for s in seg_sub:
        for k in range(0, s, 2):
            legal.add(pos + k)
        legal.add(pos + s)
        pos += s

    # target cumulative boundaries as fractions of the 130-subchunk shape
    frac = [6 / 130, 15 / 130, 26 / 130, 43 / 130, 65 / 130, 91 / 130, 117 / 130]
    cum = []
    prev = 0
    for f in frac:
        want = int(round(f * total))
        # snap down to a legal boundary > prev
        b = want
        while b > prev and b not in legal:
            b -= 1
        if b <= prev:
            b = want
            while b < total and b not in legal:
                b += 1
        if prev < b < total:
            cum.append(b)
            prev = b
    cum.append(total)
    sizes = [cum[0]] + [cum[i] - cum[i - 1] for i in range(1, len(cum))]
    return [s for s in sizes if s > 0]


def _build_bass_program(seg_sub):
    """SPMD bass program computing per-class partial grams.

    seg_sub: list of C ints — 128-row subchunks per class (identical on all
    cores; zero padded on the host). Device input "z" is the class-sorted,
    padded, PRE-TILED Z in fp8e4m3: for each DMA tile t of tsz subchunks, a
    contiguous [128, tsz*128] block (each SBUF partition's data contiguous
    in DRAM). Output "m_out": [128, C*128] fp16 partial M."""
    import concourse.bacc as bacc
    import concourse.tile as tile
    from concourse import mybir
    from contextlib import ExitStack

    total_sub = sum(seg_sub)
    tile_sizes = _dma_tile_sizes(seg_sub)
    plan, _, _ = _matmul_plan(seg_sub)

    # bank groups: classes [0..3] / [4..7] / [8] / [9]; separate PSUM tiles
    # so each group's drain depends only on that group's matmuls; class 9
    # never touches SBUF — its bank goes straight to DRAM at the end
    groups = [(0, 4), (4, 8), (8, 9), (9, C)]

    nc = bacc.Bacc("TRN2", target_bir_lowering=False, debug=False, num_devices=N_CORES)
    z = nc.dram_tensor(
        "z", [total_sub * CHUNK, D], mybir.dt.float8e4, kind="ExternalInput"
    ).ap()
    # fp16 partials: |entry| <= ~2.5k fits easily, the 2^-11 rounding is far
    # below the fp8-input noise floor, and the store bytes halve
    out = nc.dram_tensor(
        "m_out", [D, C * D], mybir.dt.float16, kind="ExternalOutput"
    ).ap()

    # tile 0 loads via a RAW pre-context DMA: the TileContext body only
    # starts after a ~0.9us mini-barrier + branch, but the DMA rings are
    # configured well before that, so issuing the first tile's transfer
    # ahead of the context starts its flight ~1.5us earlier. The PE is
    # gated on the manual completion semaphore via a throwaway matmul
    # (waits attached to a self-loading matmul land on the MATMUL half of
    # the lowered LDWEIGHTS+MATMUL pair, so the wait must ride an earlier
    # Tensor-queue instruction to also gate the first real weight load).
    t0sz = tile_sizes[0]
    t1sz = tile_sizes[1]
    z0 = nc.alloc_sbuf_tensor("z0raw", [128, t0sz, D], mybir.dt.float8e4)
    z1 = nc.alloc_sbuf_tensor("z1raw", [128, t1sz, D], mybir.dt.float8e4)
    z0sem = nc.alloc_semaphore("z0sem")
    z1sem = nc.alloc_semaphore("z1sem")
    nc.sync.dma_start(
        z0.ap(), z[0 : CHUNK * t0sz, :].rearrange("(p k) d -> p k d", p=128)
    ).then_inc(z0sem, 16)
    nc.scalar.dma_start(
        z1.ap(),
        z[CHUNK * t0sz : CHUNK * (t0sz + t1sz), :].rearrange(
            "(p k) d -> p k d", p=128
        ),
    ).then_inc(z1sem, 16)

    with tile.TileContext(nc) as tc:
        with ExitStack() as ctx:
            psum = ctx.enter_context(tc.tile_pool(name="psum", bufs=1, space="PSUM"))
            opool = ctx.enter_context(tc.tile_pool(name="o", bufs=1))
            accs = [
                psum.tile([128, (hi - lo) * D], mybir.dt.float32, name=f"acc{gi}")
                for gi, (lo, hi) in enumerate(groups)
            ]
            scratch = psum.tile([128, 1], mybir.dt.float32)
            sb_out = opool.tile([128, C * D], mybir.dt.float16)
            # the gate: loads garbage weights ungated (never read), then its
            # MATMUL carries the z0sem wait (attached post-scheduling) and,
            # because the Tensor queue is in-order, blocks every later
            # LDWEIGHTS until tile 0's data has landed
            gate_mm = nc.tensor.matmul(
                scratch[:],
                z0.ap()[:, 0:2, :],
                z0.ap()[:, 0:2, 0:1],
                start=True,
                stop=True,
                perf_mode=mybir.MatmulPerfMode.DoubleRow,
                skip_group_check=True,
            )
            pi = 0  # next matmul in plan
            row0 = 0
            gate2 = None
            for t, tsz in enumerate(tile_sizes):
                if t == 0:
                    tl = z0.ap()
                elif t == 1:
                    tl = z1.ap()
                    # second gate: blocks tile 1's weight loads until its
                    # raw DMA lands (in-order Tensor queue)
                    gate2 = nc.tensor.matmul(
                        scratch[:],
                        z1.ap()[:, 0:2, :],
                        z1.ap()[:, 0:2, 0:1],
                        start=True,
                        stop=True,
                        perf_mode=mybir.MatmulPerfMode.DoubleRow,
                        skip_group_check=True,
                    )
                else:
                    pool = ctx.enter_context(tc.tile_pool(name=f"z{t}", bufs=1))
                    tl = pool.tile([128, tsz, D], mybir.dt.float8e4)
                    src = z[row0 : row0 + CHUNK * tsz, :].rearrange(
                        "(p k) d -> p k d", p=128
                    )
                    # the two HWDGE rings alternate strictly (sync picks up
                    # ~0.6us sooner after the preamble, so it takes the
                    # even tiles); SWDGE/gpsimd stays out of the data path
                    eng = nc.sync if t % 2 == 0 else nc.scalar
                    eng.dma_start(tl[:], src)
                tile_lo = row0 // CHUNK
                row0 += CHUNK * tsz
                while pi < len(plan) and plan[pi][0] + plan[pi][1] <= tile_lo + tsz:
                    pos, sz, j, is_first, is_last = plan[pi]
                    k = pos - tile_lo
                    g = next(gi for gi, (lo_, hi_) in enumerate(groups) if lo_ <= j < hi_)
                    lo = groups[g][0]
                    acc = accs[g]
                    sl = tl[:, k : k + sz, :]
                    nc.tensor.matmul(
                        acc[:, (j - lo) * D : (j - lo + 1) * D],
                        sl,
                        sl,
                        start=is_first,
                        stop=is_last,
                        perf_mode=(
                            mybir.MatmulPerfMode.DoubleRow if sz == 2 else None
                        ),
                        skip_group_check=True,
                    )
                    # drain finished PSUM bank groups so the DVE read never
                    # shares a bank with in-flight PE writes
                    if is_last and j == groups[g][1] - 1:
                        sl_o = slice(lo * D, groups[g][1] * D)
                        nc.vector.tensor_copy(sb_out[:, sl_o], acc[:])
                        # classes 0..7 merge into one scalar-ring store as
                        # soon as class 7's copy lands; class 8 rides the
                        # scalar ring too (completes while class 9's
                        # matmuls still run); only class 9's 256B/partition
                        # store trails the last matmul, on the sync ring
                        if g == 1:
                            nc.scalar.dma_start(
                                out[:, 0 : 8 * D], sb_out[:, 0 : 8 * D]
                            )
                        elif g == 2:
                            nc.scalar.dma_start(out[:, sl_o], sb_out[:, sl_o])
                        elif g == 3:
                            nc.sync.dma_start(out[:, sl_o], sb_out[:, sl_o])
                    pi += 1
    # attach the gates' waits AFTER the tile scheduler ran (its simulator
    # can't see the external DMAs and would report a deadlock), then reset
    # the manual semaphores so back-to-back NEFF executions start from zero
    gate_mm.wait_op(z0sem, 16, "sem-ge")
    gate2.wait_op(z1sem, 16, "sem-ge")
    nc.gpsimd.sem_clear(z0sem)
    nc.gpsimd.sem_clear(z1sem)
    assert pi == len(plan)
    nc.compile()
    return nc


def _is_one_hot(Pi):
    if not (Pi.sum(axis=1) == 1.0).all():
        return False
    if not (Pi.max(axis=1) == 1.0).all():
        return False
    return np.count_nonzero(Pi) == Pi.shape[0]


def _fast_path_M(Z, Pi):
    """Per-class second moments via the device. Returns M [C, D, D] fp64."""
    import ml_dtypes
    from concourse.bass_utils import run_bass_kernel_spmd

    labels = np.argmax(Pi, axis=1)

    # balance every class across cores: class j's rows are dealt out in
    # near-equal contiguous slices, so per-class per-core counts differ by
    # at most 1 and padding is minimal
    order = np.argsort(labels, kind="stable")
    cls_counts = np.bincount(labels, minlength=C)
    cls_offs = np.concatenate([[0], np.cumsum(cls_counts)])

    counts = np.zeros((N_CORES, C), dtype=np.int64)
    for j in range(C):
        m = cls_counts[j]
        base, rem = divmod(m, N_CORES)
        for c in range(N_CORES):
            counts[c, j] = base + (1 if c < rem else 0)

    seg_sub = [max(1, int(np.ceil(counts[:, j].max() / CHUNK))) for j in range(C)]
    total_sub = sum(seg_sub)
    tile_sizes = _dma_tile_sizes(seg_sub)
    _, plain_pos, dr_pos = _matmul_plan(seg_sub)

    key = tuple(seg_sub)
    if key not in _compiled_cache:
        _compiled_cache[key] = _build_bass_program(seg_sub)
    nc = _compiled_cache[key]

    # ship fp8e4m3: quarters HBM traffic vs fp32 and doubles PE throughput
    # via DoubleRow; the rounding effect on the final losses is ~1.6e-3
    # relative (measured), an order of magnitude inside the gate
    Zb = Z.astype(ml_dtypes.float8_e4m3)
    in_maps = []
    for c in range(N_CORES):
        zbuf = np.zeros((total_sub * CHUNK, D), dtype=ml_dtypes.float8_e4m3)
        for j in range(C):
            lo = cls_offs[j] + counts[:c, j].sum()
            nj = counts[c, j]
            rows = Zb[order[lo : lo + nj]]
            if j in plain_pos:
                take = min(nj, CHUNK)
                p0 = plain_pos[j] * CHUNK
                zbuf[p0 : p0 + take] = rows[:take]
                rows = rows[take:]
            d0 = dr_pos[j] * CHUNK
            zbuf[d0 : d0 + len(rows)] = rows
        # pre-tile each DMA block: [tsz, 128, D] -> [128, tsz*D]
        parts = []
        start = 0
        for tsz in tile_sizes:
            blk = zbuf[start * CHUNK : (start + tsz) * CHUNK]
            parts.append(
                np.ascontiguousarray(
                    blk.reshape(tsz, CHUNK, D).transpose(1, 0, 2)
                ).reshape(-1)
            )
            start += tsz
        zdev = np.concatenate(parts).reshape(total_sub * CHUNK, D)
        in_maps.append({"z": zdev})

    res = run_bass_kernel_spmd(nc, in_maps, list(range(N_CORES)))
    M = np.zeros((C, D, D), dtype=np.float64)
    for c in range(N_CORES):
        o = res.results[c]["m_out"].astype(np.float64)  # [D, C*D]
        M += o.reshape(D, C, D).transpose(1, 0, 2)
    return M


def _dense_path_M(Z, Pi):
    """General dense Pi: host BLAS contraction. Returns (M, gram) fp64."""
    Zf = np.ascontiguousarray(Z, dtype=np.float32)
    A = (Pi[:, :, None].astype(np.float32) * Zf[:, None, :]).reshape(Zf.shape[0], -1)
    M = (A.T @ Zf).reshape(C, D, D).astype(np.float64)
    gram = (Zf.T @ Zf).astype(np.float64)
    return M, gram


def kernel(Z, Pi, Us):
    Z = np.asarray(Z, dtype=np.float32)
    Pi = np.asarray(Pi, dtype=np.float32)
    Us = np.asarray(Us, dtype=np.float32)
    n, d = Z.shape

    if n == N_TOTAL and d == D and Pi.shape == (n, C) and _is_one_hot(Pi):
        M = _fast_path_M(Z, Pi)
        gram = M.sum(axis=0)
    else:
        M, gram = _dense_path_M(Z, Pi)

    nf = float(n)
    df = float(d)

    A = np.eye(d, dtype=np.float64) + (df / (nf * EPS)) * gram
    sign, logabsdet = np.linalg.slogdet(A)
    loss_R = 0.5 * logabsdet

    trPi = Pi.astype(np.float64).sum(axis=0)
    col_norms_sq = (Us.astype(np.float64) ** 2).sum(axis=1)  # [C, d]
    with np.errstate(divide="ignore"):
        per_class = np.log1p((df / (trPi[:, None] * EPS)) * col_norms_sq).sum(axis=1)
    loss_Rc = ((trPi / (2.0 * nf)) * per_class).sum()

    Us64 = Us.astype(np.float64)
    UUt = np.einsum("jdk,jek->jde", Us64, Us64)
    loss_reg = 0.5 * MU * ((M - UUt) ** 2).sum()

    loss_obj = loss_R - loss_Rc - loss_reg
    return (
        np.float32(-loss_obj),
        np.float32(loss_R),
        np.float32(loss_Rc),
        np.float32(loss_reg),
    )


# revision 9
# speedup vs baseline: 1.0264x; 1.0264x over previous
"""MCR2 variational loss on 8 Trainium2 NeuronCores.

Strategy (data-parallel over the sample axis n):
  - The heavy part of the loss is the per-class second-moment matrices
    M_j = Z^T diag(Pi_j) Z (plus the global gram Z^T Z), which reads all of
    Z/Pi once -> memory-bound. Everything downstream (logdet, log1p terms,
    Frobenius distance) is O(C*d^2) scalar work done on the host in fp64.
  - Fast path (Pi exactly one-hot): each sample contributes to exactly one
    class, so per-class partial grams over class-sorted rows give all M_j,
    and gram = sum_j M_j. Host deals each class's rows out to the 8 cores,
    keeps an EVEN number of full 128-row subchunks per class on the device
    (no padding, no odd trailing chunk -> pure DoubleRow fp8e4m3 matmuls,
    256 rows per instruction), and absorbs the ~100 leftover rows per
    (core, class) into an exact fp32 Gram on the host. fp8 keeps the final
    losses within ~1.6e-3 relative (measured), far inside the 2e-2 gate.
  - The PE runs at a fixed DVFS-throttled cadence (~127ns per DoubleRow
    unit) until an absolute ~18us mark, so the kernel's job is to start
    the matmul stream as early as possible and never stall it: the first
    two DMA issues are hoisted BEFORE the framework's init barrier (the
    rings are configured at NEFF load, so the transfers fly while the
    engines still rendezvous), and tiles ramp 4->24 subchunks so delivery
    (~250-300GB/s aggregate; the 16 SDMA engines round-robin both HWDGE
    rings' packets) stays ahead of the PE's ~258GB/s throttled burn.
  - Output: classes 0-3 / 4-7 / 8 drain PSUM->SBUF (fp16) and store on
    the scalar ring while the PE still works; only class 9's
    256B/partition store (sync ring) trails the last matmul.
  - Fallback (general dense Pi): host BLAS contraction.
"""

import numpy as np

EPS = 0.5
MU = 1.0
C = 10
N_TOTAL = 131072
D = 128
N_CORES = 8
CHUNK = 128  # rows per subchunk (PE partition/contraction dim)

_compiled_cache = {}


def _matmul_plan(seg_sub):
    """Per-class unit decomposition: DoubleRow units of 2 subchunks (all
    class lengths are even on the fast path), plus a plain single-subchunk
    matmul for any odd class length (fallback only). Each class's PSUM
    accumulation group stays CONTIGUOUS in the instruction stream —
    interleaving groups corrupts earlier partial sums on hardware.

    Returns (plan, plain_pos, dr_pos): plan is a position-sorted list of
    (global_subchunk_pos, size, class, is_first, is_last)."""
    plain_pos = {}
    dr_pos = {}
    plan = []
    pos = 0
    for j, s in enumerate(seg_sub):
        ndr = s // 2
        dr_pos[j] = pos
        for u in range(ndr):
            plan.append((pos + 2 * u, 2, j, u == 0, s % 2 == 0 and u == ndr - 1))
        if s % 2 == 1:
            plain_pos[j] = pos + 2 * ndr
            plan.append((plain_pos[j], 1, j, ndr == 0, True))
        pos += s
    return plan, plain_pos, dr_pos


def _dma_tile_sizes(seg_sub):
    """Ramped tile sizes (in subchunks). Boundaries never split a DoubleRow
    unit: legal cut points within class c (start s0, length s) are
    s0+{0,2,..} and s0+s. Small first tiles for PE start latency, ~3KB
    descriptors mid-stream for DMA bandwidth, smallish last tile so the PE
    tail isn't gated on a monster transfer."""
    total = sum(seg_sub)
    legal = set()
    pos = 0
    for s in seg_sub:
        for k in range(0, s, 2):
            legal.add(pos + k)
        legal.add(pos + s)
        pos += s

    ramp = [4, 8, 12, 16, 20, 24]
    sizes = []
    prev = 0
    ri = 0
    while prev < total:
        want = ramp[min(ri, len(ramp) - 1)]
        ri += 1
        b = min(prev + want, total)
        while b > prev and b not in legal:
            b -= 1
        if b <= prev:
            b = prev + want
            while b < total and b not in legal:
                b += 1
            b = min(b, total)
        sizes.append(b - prev)
        prev = b
    # keep the final tile modest: if the last tile ended up tiny (<4) fold
    # it back is not possible w/o resplit; fine either way
    return sizes


def _build_bass_program(seg_sub):
    """SPMD bass program computing per-class partial grams.

    seg_sub: list of C ints — 128-row subchunks per class (identical on all
    cores). Device input "z" is the class-sorted, PRE-TILED Z in fp8e4m3:
    for each DMA tile t of tsz subchunks, a contiguous [128, tsz*128]
    block (each SBUF partition's data contiguous in DRAM). Output "m_out":
    [128, C*128] fp16 partial M."""
    import concourse.bacc as bacc
    import concourse.tile as tile
    from concourse import mybir
    from contextlib import ExitStack

    total_sub = sum(seg_sub)
    tile_sizes = _dma_tile_sizes(seg_sub)
    plan, _, _ = _matmul_plan(seg_sub)

    # bank groups: classes [0..3] / [4..7] / [8] / [9]; separate PSUM tiles
    # so each group's drain depends only on that group's matmuls
    groups = [(0, 4), (4, 8), (8, 9), (9, C)]

    nc = bacc.Bacc("TRN2", target_bir_lowering=False, debug=False, num_devices=N_CORES)
    z = nc.dram_tensor(
        "z", [total_sub * CHUNK, D], mybir.dt.float8e4, kind="ExternalInput"
    ).ap()
    # fp16 partials: |entry| <= ~2.5k fits easily, the 2^-11 rounding is far
    # below the fp8-input noise floor, and the store bytes halve
    out = nc.dram_tensor(
        "m_out", [D, C * D], mybir.dt.float16, kind="ExternalOutput"
    ).ap()

    # tiles 0/1 load via RAW pre-context DMAs whose issue instructions are
    # hoisted BEFORE the framework's init barrier (below): the DMA rings
    # are configured at NEFF load, so the transfers are in flight while
    # the engines still rendezvous/memset. The PE is gated on the manual
    # completion semaphores via throwaway matmuls (waits attached to a
    # self-loading matmul land on the MATMUL half of the lowered
    # LDWEIGHTS+MATMUL pair, so the wait must ride an earlier Tensor-queue
    # instruction to also gate the first real weight load).
    t0sz = tile_sizes[0]
    t1sz = tile_sizes[1]
    z0 = nc.alloc_sbuf_tensor("z0raw", [128, t0sz, D], mybir.dt.float8e4)
    z1 = nc.alloc_sbuf_tensor("z1raw", [128, t1sz, D], mybir.dt.float8e4)
    z0sem = nc.alloc_semaphore("z0sem")
    z1sem = nc.alloc_semaphore("z1sem")
    dma0 = nc.sync.dma_start(
        z0.ap(), z[0 : CHUNK * t0sz, :].rearrange("(p k) d -> p k d", p=128)
    ).then_inc(z0sem, 16)
    dma1 = nc.scalar.dma_start(
        z1.ap(),
        z[CHUNK * t0sz : CHUNK * (t0sz + t1sz), :].rearrange(
            "(p k) d -> p k d", p=128
        ),
    ).then_inc(z1sem, 16)
    # hoist the two issues to right after their engine's register preamble,
    # ahead of the framework's const-memset barrier: saves ~1.2us of issue
    # latency and the issue slips before first_useful.
    entry = nc.main_func.blocks[0]
    il = entry.instructions
    for eng, bi in ((nc.sync, dma0), (nc.scalar, dma1)):
        il.remove(bi.ins)
        il.insert(il.index(eng.preamble_end) + 1, bi.ins)

    with tile.TileContext(nc) as tc:
        with ExitStack() as ctx:
            psum = ctx.enter_context(tc.tile_pool(name="psum", bufs=1, space="PSUM"))
            opool = ctx.enter_context(tc.tile_pool(name="o", bufs=1))
            accs = [
                psum.tile([128, (hi - lo) * D], mybir.dt.float32, name=f"acc{gi}")
                for gi, (lo, hi) in enumerate(groups)
            ]
            scratch = psum.tile([128, 1], mybir.dt.float32)
            sb_out = opool.tile([128, C * D], mybir.dt.float16)
            # the gate: loads garbage weights ungated (never read), then its
            # MATMUL carries the z0sem wait (attached post-scheduling) and,
            # because the Tensor queue is in-order, blocks every later
            # LDWEIGHTS until tile 0's data has landed
            gate_mm = nc.tensor.matmul(
                scratch[:],
                z0.ap()[:, 0:2, :],
                z0.ap()[:, 0:2, 0:1],
                start=True,
                stop=True,
                perf_mode=mybir.MatmulPerfMode.DoubleRow,
                skip_group_check=True,
            )
            pi = 0  # next matmul in plan
            row0 = 0
            gate2 = None
            for t, tsz in enumerate(tile_sizes):
                if t == 0:
                    tl = z0.ap()
                elif t == 1:
                    tl = z1.ap()
                    # second gate: blocks tile 1's weight loads until its
                    # raw DMA lands (in-order Tensor queue)
                    gate2 = nc.tensor.matmul(
                        scratch[:],
                        z1.ap()[:, 0:2, :],
                        z1.ap()[:, 0:2, 0:1],
                        start=True,
                        stop=True,
                        perf_mode=mybir.MatmulPerfMode.DoubleRow,
                        skip_group_check=True,
                    )
                else:
                    pool = ctx.enter_context(tc.tile_pool(name=f"z{t}", bufs=1))
                    tl = pool.tile([128, tsz, D], mybir.dt.float8e4)
                    src = z[row0 : row0 + CHUNK * tsz, :].rearrange(
                        "(p k) d -> p k d", p=128
                    )
                    # the two HWDGE rings alternate strictly; the 16 SDMA
                    # engines round-robin both rings' packets so two tiles
                    # are always in flight. SWDGE/gpsimd stays out of the
                    # data path (keeps the Pool exit drain cheap).
                    eng = nc.sync if t % 2 == 0 else nc.scalar
                    eng.dma_start(tl[:], src)
                tile_lo = row0 // CHUNK
                row0 += CHUNK * tsz
                while pi < len(plan) and plan[pi][0] + plan[pi][1] <= tile_lo + tsz:
                    pos, sz, j, is_first, is_last = plan[pi]
                    k = pos - tile_lo
                    g = next(gi for gi, (lo_, hi_) in enumerate(groups) if lo_ <= j < hi_)
                    lo = groups[g][0]
                    acc = accs[g]
                    sl = tl[:, k : k + sz, :]
                    nc.tensor.matmul(
                        acc[:, (j - lo) * D : (j - lo + 1) * D],
                        sl,
                        sl,
                        start=is_first,
                        stop=is_last,
                        perf_mode=(
                            mybir.MatmulPerfMode.DoubleRow if sz == 2 else None
                        ),
                        skip_group_check=True,
                    )
                    # drain finished PSUM bank groups so the DVE read never
                    # shares a bank with in-flight PE writes; stores spread
                    # over the stream: g0/g1/g2 on the scalar ring (done
                    # before the exit barrier), only class 9's tiny store
                    # (sync ring) trails the last matmul
                    if is_last and j == groups[g][1] - 1:
                        sl_o = slice(lo * D, groups[g][1] * D)
                        nc.vector.tensor_copy(sb_out[:, sl_o], acc[:])
                        if g == 3:
                            nc.sync.dma_start(out[:, sl_o], sb_out[:, sl_o])
                        else:
                            nc.scalar.dma_start(out[:, sl_o], sb_out[:, sl_o])
                    pi += 1
    # attach the gates' waits AFTER the tile scheduler ran (its simulator
    # can't see the external DMAs and would report a deadlock), then reset
    # the manual semaphores so back-to-back NEFF executions start from zero
    gate_mm.wait_op(z0sem, 16, "sem-ge")
    gate2.wait_op(z1sem, 16, "sem-ge")
    nc.gpsimd.sem_clear(z0sem)
    nc.gpsimd.sem_clear(z1sem)
    assert pi == len(plan)
    nc.compile()
    return nc


def _is_one_hot(Pi):
    if not (Pi.sum(axis=1) == 1.0).all():
        return False
    if not (Pi.max(axis=1) == 1.0).all():
        return False
    return np.count_nonzero(Pi) == Pi.shape[0]


def _fast_path_M(Z, Pi):
    """Per-class second moments via the device plus an exact host Gram of
    the leftover rows. Returns M [C, D, D] fp64."""
    import ml_dtypes
    from concourse.bass_utils import run_bass_kernel_spmd

    labels = np.argmax(Pi, axis=1)

    # deal each class's rows out to cores in near-equal contiguous slices
    order = np.argsort(labels, kind="stable")
    cls_counts = np.bincount(labels, minlength=C)
    cls_offs = np.concatenate([[0], np.cumsum(cls_counts)])

    counts = np.zeros((N_CORES, C), dtype=np.int64)
    for j in range(C):
        m = cls_counts[j]
        base, rem = divmod(m, N_CORES)
        for c in range(N_CORES):
            counts[c, j] = base + (1 if c < rem else 0)

    # device takes an EVEN number of full subchunks per class (pure
    # DoubleRow, zero padding); leftovers go to an exact host-side Gram
    seg_sub = []
    for j in range(C):
        s = int(counts[:, j].min()) // CHUNK
        s -= s % 2
        seg_sub.append(s)

    if min(seg_sub) < 2:
        return None  # degenerate split; caller falls back to dense path

    total_sub = sum(seg_sub)
    tile_sizes = _dma_tile_sizes(seg_sub)
    _, plain_pos, dr_pos = _matmul_plan(seg_sub)

    key = tuple(seg_sub)
    if key not in _compiled_cache:
        _compiled_cache[key] = _build_bass_program(seg_sub)
    nc = _compiled_cache[key]

    # ship fp8e4m3: quarters HBM traffic vs fp32 and doubles PE throughput
    # via DoubleRow; rounding effect on the final losses ~1.6e-3 relative
    Zb = Z.astype(ml_dtypes.float8_e4m3)
    M = np.zeros((C, D, D), dtype=np.float64)
    in_maps = []
    rem_rows = {j: [] for j in range(C)}
    for c in range(N_CORES):
        zbuf = np.zeros((total_sub * CHUNK, D), dtype=ml_dtypes.float8_e4m3)
        for j in range(C):
            lo = cls_offs[j] + counts[:c, j].sum()
            nj = counts[c, j]
            ndev = seg_sub[j] * CHUNK
            idx = order[lo : lo + nj]
            d0 = dr_pos[j] * CHUNK
            zbuf[d0 : d0 + ndev] = Zb[idx[:ndev]]
            if nj > ndev:
                rem_rows[j].append(idx[ndev:])
        # pre-tile each DMA block: [tsz, 128, D] -> [128, tsz*D]
        parts = []
        start = 0
        for tsz in tile_sizes:
            blk = zbuf[start * CHUNK : (start + tsz) * CHUNK]
            parts.append(
                np.ascontiguousarray(
                    blk.reshape(tsz, CHUNK, D).transpose(1, 0, 2)
                ).reshape(-1)
            )
            start += tsz
        zdev = np.concatenate(parts).reshape(total_sub * CHUNK, D)
        in_maps.append({"z": zdev})

    # exact fp32 Gram of the leftover rows, overlapped-none (cheap: ~8k rows)
    for j in range(C):
        if rem_rows[j]:
            idx = np.concatenate(rem_rows[j])
            Lj = Z[idx].astype(np.float32)
            M[j] += (Lj.T @ Lj).astype(np.float64)

    res = run_bass_kernel_spmd(nc, in_maps, list(range(N_CORES)))
    for c in range(N_CORES):
        o = res.results[c]["m_out"].astype(np.float64)  # [D, C*D]
        M += o.reshape(D, C, D).transpose(1, 0, 2)
    return M


def _dense_path_M(Z, Pi):
    """General dense Pi: host BLAS contraction. Returns (M, gram) fp64."""
    Zf = np.ascontiguousarray(Z, dtype=np.float32)
    A = (Pi[:, :, None].astype(np.float32) * Zf[:, None, :]).reshape(Zf.shape[0], -1)
    M = (A.T @ Zf).reshape(C, D, D).astype(np.float64)
    gram = (Zf.T @ Zf).astype(np.float64)
    return M, gram


def kernel(Z, Pi, Us):
    Z = np.asarray(Z, dtype=np.float32)
    Pi = np.asarray(Pi, dtype=np.float32)
    Us = np.asarray(Us, dtype=np.float32)
    n, d = Z.shape

    M = None
    if n == N_TOTAL and d == D and Pi.shape == (n, C) and _is_one_hot(Pi):
        M = _fast_path_M(Z, Pi)
    if M is not None:
        gram = M.sum(axis=0)
    else:
        M, gram = _dense_path_M(Z, Pi)

    nf = float(n)
    df = float(d)

    A = np.eye(d, dtype=np.float64) + (df / (nf * EPS)) * gram
    sign, logabsdet = np.linalg.slogdet(A)
    loss_R = 0.5 * logabsdet

    trPi = Pi.astype(np.float64).sum(axis=0)
    col_norms_sq = (Us.astype(np.float64) ** 2).sum(axis=1)  # [C, d]
    with np.errstate(divide="ignore"):
        per_class = np.log1p((df / (trPi[:, None] * EPS)) * col_norms_sq).sum(axis=1)
    loss_Rc = ((trPi / (2.0 * nf)) * per_class).sum()

    Us64 = Us.astype(np.float64)
    UUt = np.einsum("jdk,jek->jde", Us64, Us64)
    loss_reg = 0.5 * MU * ((M - UUt) ** 2).sum()

    loss_obj = loss_R - loss_Rc - loss_reg
    return (
        np.float32(-loss_obj),
        np.float32(loss_R),
        np.float32(loss_Rc),
        np.float32(loss_reg),
    )
